# revision 1
# baseline (speedup 1.0000x reference)
"""MemoryBank.get_all_distances Trainium2 kernel.

emb_batch [64, 64] f32, bank [131072, 64] f32 -> distances [64, 131072] f32
  distances[n, b] = || bank[b] - emb[n] ||_2

Strategy: shard bank rows across 8 cores (16384 rows each). On the host we
only re-layout (transpose + stack) the shard; all arithmetic runs on device:

  dist^2[n, b] = ||e_n||^2 - 2 e_n . b_b + ||b_b||^2

Per core the shard is fed as bt [128, 8192] f32: partitions 0-63 hold dim d
of bank columns 0..8191 of the shard, partitions 64-127 hold dim d of columns
8192..16383 (so DMA uses all 128 partitions at full bandwidth). Per
512-column block the PE runs two K=128/M=128 matmuls with block-diagonal
stationaries, accumulating in one PSUM bank:

  psum = [[-2*embT,0],[0,-2*embT]]^T . bt  +  [[1,0],[0,1]]^T . (bt*bt)

The vector engine squares the bank chunks, and the scalar engine finishes
with sqrt(psum + bias) where bias[n] = ||e_n||^2 (computed on device via a
ones matmul over squared embT). Matmuls use float32r (fast fp32 mode).
"""

import numpy as np

BANK = 131072
DIM = 64
BATCH = 64
N_CORES = 8
SHARD = BANK // N_CORES  # 16384 bank rows per core
HALF = SHARD // 2  # 8192 columns per partition-half
W = 2048  # DMA / DVE chunk width
NBLK = 512  # matmul moving block / psum tile width

_cache = {}

# test.py reads this after calling kernel() to get profiling info.
last_run = None


def _build(half=HALF, w=W, nblk=NBLK):
    import concourse.mybir as mybir
    import concourse.tile as tile
    from concourse import bacc

    f32 = mybir.dt.float32
    f32r = mybir.dt.float32r
    SQRT = mybir.ActivationFunctionType.Sqrt

    nc = bacc.Bacc(
        "TRN2", target_bir_lowering=False, debug=False, num_devices=N_CORES
    )
    bt = nc.dram_tensor("bt", [128, half], f32r, kind="ExternalInput").ap()
    ew = nc.dram_tensor("ew", [128, 128], f32, kind="ExternalInput").ap()
    o = nc.dram_tensor("o", [128, half], f32, kind="ExternalOutput").ap()

    with tile.TileContext(nc) as tc:
        with (
            tc.tile_pool(name="singles", bufs=1) as singles,
            tc.tile_pool(name="bt_pool", bufs=4) as bt_pool,
            tc.tile_pool(name="main", bufs=3) as main,
            tc.tile_pool(name="psum", bufs=6, space="PSUM") as psum,
            tc.tile_pool(name="psum_b", bufs=1, space="PSUM") as psum_b,
        ):
            # --- one-time setup -------------------------------------------
            ew2 = singles.tile([128, 128], f32)
            # ACT HWDGE ring (idle at start) — keeps the SP ring's first
            # instruction as the first bank chunk, so the big input stream
            # gets first-byte ~0.65us earlier
            nc.scalar.dma_start(out=ew2, in_=ew)
            sq_ew = singles.tile([128, 128], f32)
            nc.vector.tensor_mul(sq_ew, ew2, ew2)

            # Block-diagonal stationaries [128, 128]: fp32r matmuls must
            # write PSUM starting at partition 0, so both column-halves are
            # handled in one K=128/M=128 matmul with block-diagonal weights.
            #   em2bd = [[-2*embT, 0], [0, -2*embT]]
            #   onesbd = [[1s, 0], [0, 1s]]
            em2bd_f = singles.tile([128, 128], f32)
            nc.vector.memset(em2bd_f, 0.0)
            nc.vector.tensor_scalar_mul(
                em2bd_f[0:64, 0:64], ew2[0:64, 0:DIM], -2.0
            )
            nc.vector.tensor_scalar_mul(
                em2bd_f[64:128, 64:128], ew2[64:128, 0:DIM], -2.0
            )
            em2bd = singles.tile([128, 128], f32r)
            nc.vector.tensor_copy(out=em2bd, in_=em2bd_f)

            onesbd_f = singles.tile([128, 128], f32)
            nc.vector.memset(onesbd_f, 0.0)
            nc.vector.memset(onesbd_f[0:64, 0:64], 1.0)
            nc.vector.memset(onesbd_f[64:128, 64:128], 1.0)
            onesbd = singles.tile([128, 128], mybir.dt.bfloat16)
            nc.vector.tensor_copy(out=onesbd, in_=onesbd_f)

            ones_b = singles.tile([128, 1], f32)
            nc.vector.memset(ones_b, 1.0)

            # bias[m] = ||e_{m%64}||^2 for all 128 partitions, via
            # ones-matmul over squared embT (exact fp32, N=1).
            ps_b = psum_b.tile([128, 1], f32)
            nc.tensor.matmul(
                ps_b[:, 0:1],
                lhsT=sq_ew[0:DIM, :],
                rhs=ones_b[0:DIM, :],
                start=True,
                stop=True,
            )
            bias = singles.tile([128, 1], f32)
            nc.vector.tensor_copy(out=bias, in_=ps_b[:, 0:1])

            # --- main pipeline --------------------------------------------
            for ci in range(half // w):
                cs = slice(ci * w, (ci + 1) * w)
                bt_c = bt_pool.tile([128, w], f32r)
                nc.sync.dma_start(out=bt_c, in_=bt[:, cs])
                sq_c = main.tile([128, w], mybir.dt.bfloat16)
                out_c = main.tile([128, w], f32)
                pss = []
                # dot matmuls depend only on bt_c — issue them all first so
                # the PE starts as soon as the chunk lands, while the DVE
                # squares the chunk concurrently (per 512 block).
                for j in range(w // nblk):
                    sl = slice(j * nblk, (j + 1) * nblk)
                    ps = psum.tile([128, nblk], f32)
                    pss.append(ps)
                    nc.tensor.matmul(
                        ps,
                        lhsT=em2bd,
                        rhs=bt_c[:, sl],
                        start=True,
                        stop=False,
                    )
                    nc.vector.tensor_mul(sq_c[:, sl], bt_c[:, sl], bt_c[:, sl])
                for j in range(w // nblk):
                    sl = slice(j * nblk, (j + 1) * nblk)
                    ps = pss[j]
                    nc.tensor.matmul(
                        ps,
                        lhsT=onesbd,
                        rhs=sq_c[:, sl],
                        start=False,
                        stop=True,
                    )
                    nc.scalar.activation(
                        out=out_c[:, sl], in_=ps, func=SQRT, bias=bias, scale=1.0
                    )
                # Outputs alternate between the SWDGE (GpSimd) queue and the
                # SP HWDGE ring (idle once the input chunks are issued) so the
                # out-only phase drains from two queues.
                if ci % 2 == 0:
                    nc.gpsimd.dma_start(out=o[:, cs], in_=out_c)
                else:
                    nc.sync.dma_start(out=o[:, cs], in_=out_c)

    nc.compile()
    return nc


def _get_nc():
    if "nc" not in _cache:
        _cache["nc"] = _build()
    return _cache["nc"]


def _prep_inputs(emb_batch, bank):
    """Host-side re-layout only (shard, transpose, stack) — no arithmetic."""
    emb_batch = np.asarray(emb_batch, dtype=np.float32)
    bank = np.asarray(bank, dtype=np.float32)
    # [128, 128]: rows 0-63 and 64-127 both embT; cols duplicated so the
    # bias matmul can produce a [128, 1] result in one shot.
    ew_host = np.ascontiguousarray(np.tile(emb_batch.T, (2, 2)))
    bankT = bank.T  # [64, BANK] view
    in_maps = []
    for c in range(N_CORES):
        sh = bankT[:, c * SHARD : (c + 1) * SHARD]
        btc = np.ascontiguousarray(
            np.concatenate([sh[:, :HALF], sh[:, HALF:]], axis=0)
        )
        in_maps.append({"bt": btc, "ew": ew_host})
    return in_maps


def kernel(emb_batch, bank):
    global last_run
    from concourse.bass_utils import run_bass_kernel_spmd

    nc = _get_nc()
    in_maps = _prep_inputs(emb_batch, bank)
    res = run_bass_kernel_spmd(nc, in_maps, core_ids=list(range(N_CORES)))
    last_run = res
    out = np.empty((BATCH, BANK), dtype=np.float32)
    for c in range(N_CORES):
        oc = res.results[c]["o"]  # [128, HALF]: rows (h*64 + n)
        out[:, c * SHARD : c * SHARD + HALF] = oc[0:64]
        out[:, c * SHARD + HALF : (c + 1) * SHARD] = oc[64:128]
    return out




# revision 2
# speedup vs baseline: 1.1880x; 1.1880x over previous
"""MemoryBank.get_all_distances Trainium2 kernel.

emb_batch [64, 64] f32, bank [131072, 64] f32 -> distances [64, 131072] f32
  distances[n, b] = || bank[b] - emb[n] ||_2

Strategy: shard bank rows across 8 cores (16384 rows each). On the host we
only re-layout (transpose + stack + bf16 cast) the shard; all arithmetic runs
on device:

  dist^2[n, b] = ||e_n||^2 - 2 e_n . b_b + ||b_b||^2

Per core the shard is fed as bt [128, 8192] bf16: partitions 0-63 hold dim d
of bank columns 0..8191 of the shard, partitions 64-127 hold dim d of columns
8192..16383 (so DMA uses all 128 partitions at full bandwidth). Per
512-column block the PE runs two K=128/M=128 bf16 matmuls with block-diagonal
stationaries, accumulating in one PSUM bank:

  psum = [[-2*embT,0],[0,-2*embT]]^T . bt  +  [[1,0],[0,1]]^T . (bt*bt)

The vector engine squares the bank chunks (bf16), and the scalar engine
finishes with sqrt(psum + bias) writing bf16, where bias[n] = ||e_n||^2 is
computed on device in f32 via a DVE square + free-dim reduce over a
query-major copy of the embeddings. bf16 end to end keeps max rel err ~5e-3,
well inside the 2e-2 gate, and halves both HBM streams.
"""

import numpy as np

BANK = 131072
DIM = 64
BATCH = 64
N_CORES = 8
SHARD = BANK // N_CORES  # 16384 bank rows per core
HALF = SHARD // 2  # 8192 columns per partition-half
W = 2048  # DMA / DVE chunk width
NBLK = 512  # matmul moving block / psum bank width
ABLK = 1024  # activation / output-DMA block width

_cache = {}

# test.py reads this after calling kernel() to get profiling info.
last_run = None


def _build(half=HALF, w=W, nblk=NBLK, ablk=ABLK):
    import concourse.mybir as mybir
    import concourse.tile as tile
    from concourse import bacc

    f32 = mybir.dt.float32
    bf16 = mybir.dt.bfloat16
    SQRT = mybir.ActivationFunctionType.Sqrt

    nc = bacc.Bacc(
        "TRN2", target_bir_lowering=False, debug=False, num_devices=N_CORES
    )
    bt = nc.dram_tensor("bt", [128, half], bf16, kind="ExternalInput").ap()
    ew = nc.dram_tensor("ew", [128, 128], f32, kind="ExternalInput").ap()
    ewt = nc.dram_tensor("ewt", [128, DIM], f32, kind="ExternalInput").ap()
    o = nc.dram_tensor("o", [128, half], bf16, kind="ExternalOutput").ap()

    with tile.TileContext(nc) as tc:
        with (
            tc.tile_pool(name="singles", bufs=1) as singles,
            tc.tile_pool(name="bt_pool", bufs=4) as bt_pool,
            tc.tile_pool(name="sq_pool", bufs=2) as sq_pool,
            tc.tile_pool(name="out_pool", bufs=2) as out_pool,
            tc.tile_pool(name="psum", bufs=2, space="PSUM") as psum,
        ):
            # --- one-time setup -------------------------------------------
            ew2 = singles.tile([128, 128], f32)
            ewt2 = singles.tile([128, DIM], f32)
            # ACT HWDGE ring (idle at start) — keeps the SP ring's first
            # instruction as the first bank chunk, so the big input stream
            # gets first-byte ~0.65us earlier
            nc.scalar.dma_start(out=ew2, in_=ew)
            nc.scalar.dma_start(out=ewt2, in_=ewt)

            # bias[m] = ||e_{m%64}||^2 on the DVE: square then free-dim sum
            # of the query-major embedding copy. f32 all the way.
            sq_ewt = singles.tile([128, DIM], f32)
            nc.vector.tensor_mul(sq_ewt, ewt2, ewt2)
            bias = singles.tile([128, 1], f32)
            nc.vector.tensor_reduce(
                out=bias,
                in_=sq_ewt,
                axis=mybir.AxisListType.X,
                op=mybir.AluOpType.add,
            )

            # Preload the Sqrt activation table while the ACT engine is
            # otherwise idle (it lazily loads ~1.3us before the first use).
            warm = singles.tile([128, 1], f32)
            nc.scalar.activation(out=warm, in_=bias, func=SQRT)

            # Block-diagonal stationaries [128, 128] bf16: both column
            # halves are handled in one K=128/M=128 matmul.
            #   em2bd = [[-2*embT, 0], [0, -2*embT]]
            #   onesbd = [[1s, 0], [0, 1s]]
            em2bd_f = singles.tile([128, 128], f32)
            nc.vector.memset(em2bd_f, 0.0)
            nc.vector.tensor_scalar_mul(
                em2bd_f[0:64, 0:64], ew2[0:64, 0:DIM], -2.0
            )
            nc.vector.tensor_scalar_mul(
                em2bd_f[64:128, 64:128], ew2[64:128, 0:DIM], -2.0
            )
            em2bd = singles.tile([128, 128], bf16)
            nc.vector.tensor_copy(out=em2bd, in_=em2bd_f)

            onesbd_f = singles.tile([128, 128], f32)
            nc.vector.memset(onesbd_f, 0.0)
            nc.vector.memset(onesbd_f[0:64, 0:64], 1.0)
            nc.vector.memset(onesbd_f[64:128, 64:128], 1.0)
            onesbd = singles.tile([128, 128], bf16)
            nc.vector.tensor_copy(out=onesbd, in_=onesbd_f)

            # --- main pipeline --------------------------------------------
            for ci in range(half // w):
                cs = slice(ci * w, (ci + 1) * w)
                bt_c = bt_pool.tile([128, w], bf16)
                nc.sync.dma_start(out=bt_c, in_=bt[:, cs])
                sq_c = sq_pool.tile([128, w], bf16)
                out_c = out_pool.tile([128, w], bf16)
                ps = psum.tile([128, w], f32)
                # dot matmuls depend only on bt_c — issue them all first so
                # the PE starts as soon as the chunk lands, while the DVE
                # squares the chunk concurrently (per 512 block).
                for j in range(w // nblk):
                    sl = slice(j * nblk, (j + 1) * nblk)
                    nc.tensor.matmul(
                        ps[:, sl],
                        lhsT=em2bd,
                        rhs=bt_c[:, sl],
                        start=True,
                        stop=False,
                    )
                    nc.vector.tensor_mul(sq_c[:, sl], bt_c[:, sl], bt_c[:, sl])
                for j in range(w // nblk):
                    sl = slice(j * nblk, (j + 1) * nblk)
                    nc.tensor.matmul(
                        ps[:, sl],
                        lhsT=onesbd,
                        rhs=sq_c[:, sl],
                        start=False,
                        stop=True,
                    )
                # sqrt + bias per ablk block, then stream the output out on
                # the two HWDGE rings (scalar ring is idle; sync ring drains
                # after the input chunks).
                for j in range(w // ablk):
                    sl = slice(j * ablk, (j + 1) * ablk)
                    gs = slice(ci * w + j * ablk, ci * w + (j + 1) * ablk)
                    nc.scalar.activation(
                        out=out_c[:, sl], in_=ps[:, sl], func=SQRT,
                        bias=bias, scale=1.0,
                    )
                    if (ci * (w // ablk) + j) % 2 == 0:
                        nc.scalar.dma_start(out=o[:, gs], in_=out_c[:, sl])
                    else:
                        nc.sync.dma_start(out=o[:, gs], in_=out_c[:, sl])

    nc.compile()
    return nc


def _get_nc():
    if "nc" not in _cache:
        _cache["nc"] = _build()
    return _cache["nc"]


def _prep_inputs(emb_batch, bank):
    """Host-side re-layout only (shard, transpose, stack, bf16 cast)."""
    import ml_dtypes

    bf16 = ml_dtypes.bfloat16
    emb_batch = np.asarray(emb_batch, dtype=np.float32)
    bank = np.asarray(bank, dtype=np.float32)
    # [128, 128]: rows 0-63 and 64-127 both embT; cols duplicated.
    ew_host = np.ascontiguousarray(np.tile(emb_batch.T, (2, 2)))
    # [128, 64] query-major copy for the on-device ||e||^2 reduce.
    ewt_host = np.ascontiguousarray(np.tile(emb_batch, (2, 1)))
    bankT = bank.T  # [64, BANK] view
    in_maps = []
    for c in range(N_CORES):
        sh = bankT[:, c * SHARD : (c + 1) * SHARD]
        btc = np.ascontiguousarray(
            np.concatenate([sh[:, :HALF], sh[:, HALF:]], axis=0)
        ).astype(bf16)
        in_maps.append({"bt": btc, "ew": ew_host, "ewt": ewt_host})
    return in_maps


def kernel(emb_batch, bank):
    global last_run
    from concourse.bass_utils import run_bass_kernel_spmd

    nc = _get_nc()
    in_maps = _prep_inputs(emb_batch, bank)
    res = run_bass_kernel_spmd(nc, in_maps, core_ids=list(range(N_CORES)))
    last_run = res
    out = np.empty((BATCH, BANK), dtype=np.float32)
    for c in range(N_CORES):
        oc = res.results[c]["o"]  # [128, HALF] bf16: rows (h*64 + n)
        oc = np.asarray(oc).astype(np.float32)
        out[:, c * SHARD : c * SHARD + HALF] = oc[0:64]
        out[:, c * SHARD + HALF : (c + 1) * SHARD] = oc[64:128]
    return out
